# revision 51
# baseline (speedup 1.0000x reference)
"""Trainium2 Bass kernel for cross-attention (efficient/linear attention variant).

Computation per batch b (fully batch-independent -> data parallel over 8 cores):
    q  = Wq @ x[b]                         # (128, N)
    kv = Wkv @ context[b].T                # (256, NCTX)
    k, v = kv[:128], kv[128:]
    q = softmax_d(q) * d**-0.5             # softmax over feature dim within head
    k = softmax_n(k)                       # softmax over sequence dim
    ctx[h] = k_h @ v_h.T                   # (32, 32) per head
    out[h] = ctx[h].T @ q_h                # (32, N)
    y = Wo @ out + bo

Strategy (v2, single fused pipeline; ACT runs only Exp in phase 1 — the 1/S
path is DVE reciprocal_approx_fast + gpsimd bf16 cast, NOT ln/exp, because
every Exp<->Ln switch costs a 1283ns ACT_TABLE_LOAD):
  - One batch per NeuronCore (8 cores), no collectives.
  - Host pre-transposes context to (512, NCTX), casts streams to bf16.
  - Phase 1 (32 iters, PE kept saturated at full clock): each iter handles one
    512-wide ctx tile (kv proj with n on partitions, exp(k) on ACT, v copy on
    DVE, single accumulating matmul onto [C | Z] via a ones column in v) AND
    one 512-wide x tile (q proj, exp, per-head sum via indicator matmul,
    ln(S*e^-4) on ACT, PE broadcast, exp(-lnS) on ACT, qn = eq*rb on DVE).
    The x-chain is software-pipelined across iters (s at i-1, bcast at i-2) so
    the in-order PE queue never stalls on ACT/DVE producers.
  - Weff fold: Weff^T = (C masked block-diag / Z)^T @ Wo^T with the bias
    folded in as a rank-1 term (rows of qn sum to exactly 4*e^4). The 128x128
    block-diagonal transpose is a DVE 32x32 stream-transpose.
  - Phase 2 (DMA-bound tail): per tile two y matmuls against Weff^T, psum->
    sbuf copies split across ACT/DVE, y DMA out.
"""

import os
import sys
from contextlib import ExitStack

import numpy as np

if "/opt/trn_rl_repo" not in sys.path:
    sys.path.insert(0, "/opt/trn_rl_repo")

import ml_dtypes

import concourse.bass as bass
from concourse import bacc
import concourse.mybir as mybir
import concourse.tile as tile
from concourse.bass_utils import run_bass_kernel_spmd

HEADS = 4
DIM_HEAD = 32
SCALE = DIM_HEAD**-0.5
B = 8
DIM = 256
N = 16384
NCTX = 16384
CDIM = 512
HID = HEADS * DIM_HEAD  # 128

BF16 = mybir.dt.bfloat16
F32 = mybir.dt.float32
EXP = mybir.ActivationFunctionType.Exp
LN = mybir.ActivationFunctionType.Ln
COPY = mybir.ActivationFunctionType.Copy

TILE_N = 512  # free-dim tile for both streams


def build_graph(n: int = N, nctx: int = NCTX) -> bass.Bass:
    global N, NCTX
    saved = (N, NCTX)
    N, NCTX = n, nctx
    try:
        return _build_graph_impl()
    finally:
        N, NCTX = saved


def _build_graph_impl() -> bass.Bass:
    nc = bacc.Bacc()

    ctxt = nc.dram_tensor("ctxt", [CDIM, NCTX], BF16, kind="ExternalInput")
    xs = nc.dram_tensor("xs", [DIM, N], BF16, kind="ExternalInput")
    wqt = nc.dram_tensor("wqt", [DIM, HID], BF16, kind="ExternalInput")
    wkvt = nc.dram_tensor("wkvt", [CDIM, 2 * HID], BF16, kind="ExternalInput")
    wot = nc.dram_tensor("wot", [HID, DIM], F32, kind="ExternalInput")
    bo4 = nc.dram_tensor("bo4", [1, DIM], F32, kind="ExternalInput")
    ind4 = nc.dram_tensor("ind4", [HID, HEADS], BF16, kind="ExternalInput")
    ind128 = nc.dram_tensor("ind128", [HEADS, HID], BF16, kind="ExternalInput")
    bmask = nc.dram_tensor("bmask", [HID, HID], F32, kind="ExternalInput")
    y = nc.dram_tensor("y", [DIM, N], BF16, kind="ExternalOutput")

    n_tiles = N // TILE_N          # 32 x tiles
    nctx_tiles = NCTX // TILE_N    # 32 ctx tiles
    assert n_tiles == nctx_tiles
    chunks = TILE_N // 128         # 4 chunks of 128 per ctx tile
    total_chunks = NCTX // 128

    ctxt_r = ctxt.rearrange("(cc p) n -> p cc n", p=128)  # (128, 4, NCTX)
    xr = xs.rearrange("(cc p) n -> p cc n", p=128)        # (128, 2, N)
    yr = y.rearrange("(oc p) n -> p oc n", p=128)         # (128, 2, N)

    with tile.TileContext(nc) as tc, ExitStack() as ctx:
        cpool = ctx.enter_context(tc.tile_pool(name="consts", bufs=1))

        # first kv matmuls need only wkvt (ACT queue) + ct(0) piece 0 (SP)
        wkvt_sb = cpool.tile([128, 4, 2 * HID], BF16)
        nc.scalar.dma_start(wkvt_sb, wkvt.rearrange("(cc p) m -> p cc m", p=128))

        weffT_sb = cpool.tile([HID, DIM], BF16)  # folded (Wo @ maskedC^T/Z)^T
        eq_all = cpool.tile([128, n_tiles, TILE_N], BF16)  # exp(q)
        rs_all = cpool.tile([HEADS, n_tiles, TILE_N], BF16)  # 1/S per head
        # qn for EVEN tiles is normalized during phase 1 (PE/DVE slack there);
        # odd tiles are normalized on the fly in phase 2 so its DVE load
        # (qn + oc1 psum copy) stays at the DMA pace
        qn_even = cpool.tile([128, n_tiles // 2, TILE_N], BF16)

        # ------- Phase 1: fused ctx->C|Z stream + x->qn stream -------------
        with (
            tc.tile_pool(name="ctp", bufs=3) as ctpool,
            tc.tile_pool(name="xtp", bufs=3) as xtpool,
            tc.tile_pool(name="kvp", bufs=2) as kvpool,
            tc.tile_pool(name="rsp", bufs=3) as rspool,
            tc.tile_pool(name="ps_kv", bufs=2, space="PSUM") as ps_kv,
            tc.tile_pool(name="ps_cz", bufs=1, space="PSUM") as ps_cz,
            tc.tile_pool(name="ps_q", bufs=1, space="PSUM") as ps_q,
            tc.tile_pool(name="ps_s", bufs=2, space="PSUM") as ps_s,
            tc.tile_pool(name="ps_rb1", bufs=1, space="PSUM") as ps_rb1,
        ):
            cz_ps = ps_cz.tile([128, HID + 1], F32)  # [C | Z] accumulator

            cts, xts = {}, {}

            def issue_loads(i, eng=None):
                if i >= n_tiles:
                    return
                eng = eng or nc.sync
                sl = slice(i * TILE_N, (i + 1) * TILE_N)
                ct = ctpool.tile([128, chunks, TILE_N], BF16, tag="ct")
                eng.dma_start(ct, ctxt_r[:, :, sl])
                xt = xtpool.tile([128, 2, TILE_N], BF16, tag="xt")
                nc.scalar.dma_start(xt, xr[:, :, sl])
                cts[i], xts[i] = ct, xt

            # ct(0) split into 128-col pieces on SP so kv(0) starts early
            ct0 = ctpool.tile([128, chunks, TILE_N], BF16, tag="ct")
            for j in range(chunks):
                nc.sync.dma_start(
                    ct0[:, :, j * 128 : (j + 1) * 128],
                    ctxt_r[:, :, j * 128 : (j + 1) * 128],
                )
            xt0 = xtpool.tile([128, 2, TILE_N], BF16, tag="xt")
            nc.scalar.dma_start(xt0, xr[:, :, 0:TILE_N])
            cts[0], xts[0] = ct0, xt0
            issue_loads(1)

            # remaining consts issued after the hot-path tiles are in flight
            # consts go through the ACT-issued queue so they don't delay
            # the hot ct/xt stream on the SP queue
            wqt_sb = cpool.tile([128, 2, HID], BF16)
            nc.scalar.dma_start(wqt_sb, wqt.rearrange("(cc p) m -> p cc m", p=128))
            ind4_sb = cpool.tile([HID, HEADS], BF16)
            nc.scalar.dma_start(ind4_sb, ind4[:, :])
            ind128_sb = cpool.tile([HEADS, HID], BF16)
            nc.scalar.dma_start(ind128_sb, ind128[:, :])
            wot_sb = cpool.tile([HID, DIM], F32)
            nc.scalar.dma_start(wot_sb, wot[:, :])
            bo4_sb = cpool.tile([1, DIM], F32)
            nc.scalar.dma_start(bo4_sb, bo4[:, :])
            bmask_sb = cpool.tile([HID, HID], F32)
            nc.scalar.dma_start(bmask_sb, bmask[:, :])
            ones1_sb = cpool.tile([1, HID], F32)
            nc.gpsimd.memset(ones1_sb, 1.0)

            for i in range(n_tiles):
                issue_loads(i + 2)

                if i < n_tiles:
                    ct = cts.pop(i)
                    # kv projection: n on partitions, 2 chunk-groups of 2
                    for g in range(chunks // 2):
                        kvt_ps = ps_kv.tile([128, 2, 2 * HID], F32, tag="kvt")
                        for j2 in range(2):
                            j = g * 2 + j2
                            for cc in range(4):
                                nc.tensor.matmul(
                                    kvt_ps[:, j2, :],
                                    ct[:, cc, j * 128 : (j + 1) * 128],
                                    wkvt_sb[:, cc, :],
                                    start=(cc == 0),
                                    stop=(cc == 3),
                                )
                        kt = kvpool.tile([128, 2, HID], BF16, tag="kt")
                        nc.scalar.activation(kt, kvt_ps[:, :, 0:HID], EXP)
                        vto = kvpool.tile([128, 2, HID + 4], BF16, tag="vto")
                        nc.gpsimd.memset(vto[:, :, HID : HID + 1], 1.0)
                        nc.vector.tensor_copy(
                            vto[:, :, 0:HID], kvt_ps[:, :, HID : 2 * HID]
                        )
                        for j2 in range(2):
                            ci = i * chunks + g * 2 + j2
                            nc.tensor.matmul(
                                cz_ps,
                                kt[:, j2, :],
                                vto[:, j2, 0 : HID + 1],
                                start=(ci == 0),
                                stop=(ci == total_chunks - 1),
                            )

                # q pipeline runs one tile ahead (q(0)+q(1) land in the
                # prologue's idle-PE window), so s(i)/bcast(i-1) chains all
                # finish inside iter 31 and the end-of-phase drain vanishes
                qts = [0, 1] if i == 0 else ([i + 1] if i + 1 < n_tiles else [])
                for t in qts:
                    xt = xts.pop(t)
                    q_ps = ps_q.tile([128, TILE_N], F32, tag="q")
                    for cc in range(2):
                        nc.tensor.matmul(
                            q_ps,
                            wqt_sb[:, cc, :],
                            xt[:, cc, :],
                            start=(cc == 0),
                            stop=(cc == 1),
                        )
                    nc.scalar.activation(eq_all[:, t, :], q_ps, EXP)

                # per-head sums + 1/S for tile i
                s_ps = ps_s.tile([HEADS, TILE_N], F32, tag="s")
                nc.tensor.matmul(
                    s_ps, ind4_sb, eq_all[:, i, :], start=True, stop=True
                )
                rsf = rspool.tile([HEADS, TILE_N], F32, tag="rsf")
                nc.vector.reciprocal_approx_fast(rsf, s_ps)
                nc.gpsimd.tensor_copy(rs_all[:, i, :], rsf)

                if i >= 1 and (i - 1) % 2 == 0:
                    # normalize EVEN tile i-1 here; odd tiles in phase 2
                    t = i - 1
                    rb_ps = ps_rb1.tile([128, TILE_N], F32, tag="rb1")
                    nc.tensor.matmul(
                        rb_ps, ind128_sb, rs_all[:, t, :], start=True, stop=True
                    )
                    nc.vector.tensor_mul(
                        qn_even[:, t // 2, :], eq_all[:, t, :], rb_ps
                    )

            # ------- Weff fold: WeffT = (C.bmask/Z)^T @ WoT + ones.bo/4 ----
            rz = kvpool.tile([128, 1], F32, tag="rz")
            nc.vector.reciprocal(rz, cz_ps[:, HID : HID + 1])
            cm0 = kvpool.tile([128, HID], F32, tag="cm0")
            nc.vector.tensor_scalar_mul(cm0, cz_ps[:, 0:HID], rz)
            cmask = kvpool.tile([128, HID], F32, tag="cmask")
            nc.vector.tensor_mul(cmask, cm0, bmask_sb)
            cmaskT = kvpool.tile([128, HID], F32, tag="cmaskT")
            # block-diagonal 32x32 in-place transposes == full C^T here
            nc.vector.transpose(cmaskT, cmask)

            weff_ps = ps_cz.tile([128, DIM], F32, tag="weff")
            nc.tensor.matmul(weff_ps, ones1_sb, bo4_sb, start=True, stop=False)
            nc.tensor.matmul(weff_ps, cmaskT, wot_sb, start=False, stop=True)
            nc.vector.tensor_copy(weffT_sb, weff_ps)

        # ------- Phase 2: bcast 1/S, qn = eq*rb, y = WeffT.T @ qn -----------
        with (
            tc.tile_pool(name="ytp", bufs=6) as ytpool,
            tc.tile_pool(name="qnp", bufs=4) as qnpool,
            tc.tile_pool(name="ps_rb", bufs=2, space="PSUM") as ps_rb,
            tc.tile_pool(name="ps_y", bufs=3, space="PSUM") as ps_y,
        ):
            qns = {}

            def bcast(t):
                # broadcast 1/S for ODD tile t (evens were done in phase 1)
                if t >= n_tiles:
                    return None
                rb_ps = ps_rb.tile([128, TILE_N], F32, tag="rb")
                nc.tensor.matmul(
                    rb_ps, ind128_sb, rs_all[:, t, :], start=True, stop=True
                )
                return rb_ps

            def qn_mul(t, rb_ps):
                if rb_ps is None:
                    return
                qn = qnpool.tile([128, TILE_N], BF16, tag="qn")
                nc.vector.tensor_mul(qn, eq_all[:, t, :], rb_ps)
                qns[t] = qn

            rb1 = bcast(1)
            qn_mul(1, rb1)
            for t in range(n_tiles):
                tsl = slice(t * TILE_N, (t + 1) * TILE_N)
                # PE: prefetch the next odd tile's broadcast ahead of y(t)
                rb_next = bcast(t + 2) if t % 2 == 1 else None
                y_ps = ps_y.tile([128, 2, TILE_N], F32, tag="y")
                qn = qns.pop(t) if t % 2 == 1 else qn_even[:, t // 2, :]
                for oc in range(2):
                    nc.tensor.matmul(
                        y_ps[:, oc, :],
                        weffT_sb[:, oc * 128 : (oc + 1) * 128],
                        qn,
                        start=True,
                        stop=True,
                    )
                yt = ytpool.tile([128, 2, TILE_N], BF16, tag="yt")
                nc.scalar.activation(yt[:, 0, :], y_ps[:, 0, :], COPY)
                nc.vector.tensor_copy(yt[:, 1, :], y_ps[:, 1, :])
                if rb_next is not None:
                    qn_mul(t + 2, rb_next)
                nc.sync.dma_start(yr[:, :, tsl], yt)

    nc.compile()
    return nc


_GRAPH_CACHE: dict = {}


def _prep_inputs(x, context, Wq, Wkv, Wo, bo):
    bf16 = ml_dtypes.bfloat16
    x = np.asarray(x, dtype=np.float32)
    context = np.asarray(context, dtype=np.float32)
    Wq = np.asarray(Wq, dtype=np.float32)
    Wkv = np.asarray(Wkv, dtype=np.float32)
    Wo = np.asarray(Wo, dtype=np.float32)
    bo = np.asarray(bo, dtype=np.float32)

    wqt = np.ascontiguousarray(Wq.T).astype(bf16)              # (256, 128)
    wkvt = np.ascontiguousarray(Wkv.T).astype(bf16)            # (512, 256)
    # SCALE folded into Wo; kept f32 for the one-shot Weff matmul
    wot = np.ascontiguousarray((Wo * SCALE).T).astype(np.float32)  # (128, 256)
    # rows of qn sum to exactly 4 -> rank-1 bias fold
    bo4 = np.ascontiguousarray((bo / 4.0)[None, :]).astype(np.float32)  # (1, 256)

    ind4 = np.zeros((HID, HEADS), dtype=bf16)
    ind4[np.arange(HID), np.arange(HID) // DIM_HEAD] = 1
    ind128 = np.ascontiguousarray(ind4.T)
    bmask = (
        (np.arange(HID)[:, None] // DIM_HEAD) == (np.arange(HID)[None, :] // DIM_HEAD)
    ).astype(np.float32)

    in_maps = []
    for b in range(B):
        in_maps.append(
            {
                "ctxt": np.ascontiguousarray(context[b].T).astype(bf16),
                "xs": x[b].astype(bf16),
                "wqt": wqt,
                "wkvt": wkvt,
                "wot": wot,
                "bo4": bo4,
                "ind4": ind4,
                "ind128": ind128,
                "bmask": bmask,
            }
        )
    return in_maps


def run(inputs: dict, trace: bool = False):
    if "nc" not in _GRAPH_CACHE:
        _GRAPH_CACHE["nc"] = build_graph()
    nc = _GRAPH_CACHE["nc"]
    in_maps = _prep_inputs(**inputs)
    res = run_bass_kernel_spmd(nc, in_maps, core_ids=list(range(B)), trace=trace)
    out = np.stack(
        [np.asarray(res.results[b]["y"], dtype=np.float32) for b in range(B)]
    )
    return out, res


def kernel(**inputs) -> np.ndarray:
    out, _ = run(inputs, trace=False)
    return out


# revision 53
# speedup vs baseline: 1.0013x; 1.0013x over previous
"""Trainium2 Bass kernel for cross-attention (efficient/linear attention variant).

Computation per batch b (fully batch-independent -> data parallel over 8 cores):
    q  = Wq @ x[b]                         # (128, N)
    kv = Wkv @ context[b].T                # (256, NCTX)
    k, v = kv[:128], kv[128:]
    q = softmax_d(q) * d**-0.5             # softmax over feature dim within head
    k = softmax_n(k)                       # softmax over sequence dim
    ctx[h] = k_h @ v_h.T                   # (32, 32) per head
    out[h] = ctx[h].T @ q_h                # (32, N)
    y = Wo @ out + bo

Strategy (v2, single fused pipeline; ACT runs only Exp in phase 1 — the 1/S
path is DVE reciprocal_approx_fast + gpsimd bf16 cast, NOT ln/exp, because
every Exp<->Ln switch costs a 1283ns ACT_TABLE_LOAD):
  - One batch per NeuronCore (8 cores), no collectives.
  - Host pre-transposes context to (512, NCTX), casts streams to bf16.
  - Phase 1 (32 iters, PE kept saturated at full clock): each iter handles one
    512-wide ctx tile (kv proj with n on partitions, exp(k) on ACT, v copy on
    DVE, single accumulating matmul onto [C | Z] via a ones column in v) AND
    one 512-wide x tile (q proj, exp, per-head sum via indicator matmul,
    ln(S*e^-4) on ACT, PE broadcast, exp(-lnS) on ACT, qn = eq*rb on DVE).
    The x-chain is software-pipelined across iters (s at i-1, bcast at i-2) so
    the in-order PE queue never stalls on ACT/DVE producers.
  - Weff fold: Weff^T = (C masked block-diag / Z)^T @ Wo^T with the bias
    folded in as a rank-1 term (rows of qn sum to exactly 4*e^4). The 128x128
    block-diagonal transpose is a DVE 32x32 stream-transpose.
  - Phase 2 (DMA-bound tail): per tile two y matmuls against Weff^T, psum->
    sbuf copies split across ACT/DVE, y DMA out.
"""

import os
import sys
from contextlib import ExitStack

import numpy as np

if "/opt/trn_rl_repo" not in sys.path:
    sys.path.insert(0, "/opt/trn_rl_repo")

import ml_dtypes

import concourse.bass as bass
from concourse import bacc
import concourse.mybir as mybir
import concourse.tile as tile
from concourse.bass_utils import run_bass_kernel_spmd

HEADS = 4
DIM_HEAD = 32
SCALE = DIM_HEAD**-0.5
B = 8
DIM = 256
N = 16384
NCTX = 16384
CDIM = 512
HID = HEADS * DIM_HEAD  # 128

BF16 = mybir.dt.bfloat16
F32 = mybir.dt.float32
EXP = mybir.ActivationFunctionType.Exp
LN = mybir.ActivationFunctionType.Ln
COPY = mybir.ActivationFunctionType.Copy

TILE_N = 512  # free-dim tile for both streams


def build_graph(n: int = N, nctx: int = NCTX) -> bass.Bass:
    global N, NCTX
    saved = (N, NCTX)
    N, NCTX = n, nctx
    try:
        return _build_graph_impl()
    finally:
        N, NCTX = saved


def _build_graph_impl() -> bass.Bass:
    nc = bacc.Bacc()

    ctxt = nc.dram_tensor("ctxt", [CDIM, NCTX], BF16, kind="ExternalInput")
    xs = nc.dram_tensor("xs", [DIM, N], BF16, kind="ExternalInput")
    wqt = nc.dram_tensor("wqt", [DIM, HID], BF16, kind="ExternalInput")
    wkvt = nc.dram_tensor("wkvt", [CDIM, 2 * HID], BF16, kind="ExternalInput")
    wot = nc.dram_tensor("wot", [HID, DIM], F32, kind="ExternalInput")
    bo4 = nc.dram_tensor("bo4", [1, DIM], F32, kind="ExternalInput")
    ind4 = nc.dram_tensor("ind4", [HID, HEADS], BF16, kind="ExternalInput")
    ind128 = nc.dram_tensor("ind128", [HEADS, HID], BF16, kind="ExternalInput")
    bmask = nc.dram_tensor("bmask", [HID, HID], F32, kind="ExternalInput")
    y = nc.dram_tensor("y", [DIM, N], BF16, kind="ExternalOutput")

    n_tiles = N // TILE_N          # 32 x tiles
    nctx_tiles = NCTX // TILE_N    # 32 ctx tiles
    assert n_tiles == nctx_tiles
    chunks = TILE_N // 128         # 4 chunks of 128 per ctx tile
    total_chunks = NCTX // 128

    ctxt_r = ctxt.rearrange("(cc p) n -> p cc n", p=128)  # (128, 4, NCTX)
    xr = xs.rearrange("(cc p) n -> p cc n", p=128)        # (128, 2, N)
    yr = y.rearrange("(oc p) n -> p oc n", p=128)         # (128, 2, N)

    with tile.TileContext(nc) as tc, ExitStack() as ctx:
        cpool = ctx.enter_context(tc.tile_pool(name="consts", bufs=1))

        # first kv matmuls need only wkvt (ACT queue) + ct(0) piece 0 (SP)
        wkvt_sb = cpool.tile([128, 4, 2 * HID], BF16)
        nc.scalar.dma_start(wkvt_sb, wkvt.rearrange("(cc p) m -> p cc m", p=128))

        weffT_sb = cpool.tile([HID, DIM], BF16)  # folded (Wo @ maskedC^T/Z)^T
        eq_all = cpool.tile([128, n_tiles, TILE_N], BF16)  # exp(q)
        rs_all = cpool.tile([HEADS, n_tiles, TILE_N], BF16)  # 1/S per head
        # qn for EVEN tiles is normalized during phase 1 (PE/DVE slack there);
        # odd tiles are normalized on the fly in phase 2 so its DVE load
        # (qn + oc1 psum copy) stays at the DMA pace
        qn_even = cpool.tile([128, n_tiles // 2, TILE_N], BF16)

        # ------- Phase 1: fused ctx->C|Z stream + x->qn stream -------------
        with (
            tc.tile_pool(name="ctp", bufs=3) as ctpool,
            tc.tile_pool(name="xtp", bufs=3) as xtpool,
            tc.tile_pool(name="kvp", bufs=2) as kvpool,
            tc.tile_pool(name="rsp", bufs=3) as rspool,
            tc.tile_pool(name="ps_kv", bufs=2, space="PSUM") as ps_kv,
            tc.tile_pool(name="ps_cz", bufs=1, space="PSUM") as ps_cz,
            tc.tile_pool(name="ps_q", bufs=1, space="PSUM") as ps_q,
            tc.tile_pool(name="ps_s", bufs=2, space="PSUM") as ps_s,
            tc.tile_pool(name="ps_rb1", bufs=1, space="PSUM") as ps_rb1,
        ):
            cz_ps = ps_cz.tile([128, HID + 1], F32)  # [C | Z] accumulator

            cts, xts = {}, {}

            def issue_loads(i, eng=None):
                if i >= n_tiles:
                    return
                eng = eng or nc.sync
                sl = slice(i * TILE_N, (i + 1) * TILE_N)
                ct = ctpool.tile([128, chunks, TILE_N], BF16, tag="ct")
                eng.dma_start(ct, ctxt_r[:, :, sl])
                xt = xtpool.tile([128, 2, TILE_N], BF16, tag="xt")
                nc.scalar.dma_start(xt, xr[:, :, sl])
                cts[i], xts[i] = ct, xt

            # ct(0) split into 128-col pieces on SP so kv(0) starts early
            ct0 = ctpool.tile([128, chunks, TILE_N], BF16, tag="ct")
            for j in range(chunks):
                nc.sync.dma_start(
                    ct0[:, :, j * 128 : (j + 1) * 128],
                    ctxt_r[:, :, j * 128 : (j + 1) * 128],
                )
            xt0 = xtpool.tile([128, 2, TILE_N], BF16, tag="xt")
            nc.scalar.dma_start(xt0, xr[:, :, 0:TILE_N])
            cts[0], xts[0] = ct0, xt0
            # ct(1) split the same way: kv(1) chunks start incrementally and
            # ct(2) queues behind smaller transfers
            ct1 = ctpool.tile([128, chunks, TILE_N], BF16, tag="ct")
            for j in range(chunks):
                nc.sync.dma_start(
                    ct1[:, :, TILE_N + j * 128 : TILE_N + (j + 1) * 128]
                    if False
                    else ct1[:, :, j * 128 : (j + 1) * 128],
                    ctxt_r[:, :, TILE_N + j * 128 : TILE_N + (j + 1) * 128],
                )
            xt1 = xtpool.tile([128, 2, TILE_N], BF16, tag="xt")
            nc.scalar.dma_start(xt1, xr[:, :, TILE_N : 2 * TILE_N])
            cts[1], xts[1] = ct1, xt1

            # remaining consts issued after the hot-path tiles are in flight
            # consts go through the ACT-issued queue so they don't delay
            # the hot ct/xt stream on the SP queue
            wqt_sb = cpool.tile([128, 2, HID], BF16)
            nc.scalar.dma_start(wqt_sb, wqt.rearrange("(cc p) m -> p cc m", p=128))
            ind4_sb = cpool.tile([HID, HEADS], BF16)
            nc.scalar.dma_start(ind4_sb, ind4[:, :])
            ind128_sb = cpool.tile([HEADS, HID], BF16)
            nc.scalar.dma_start(ind128_sb, ind128[:, :])
            wot_sb = cpool.tile([HID, DIM], F32)
            nc.scalar.dma_start(wot_sb, wot[:, :])
            bo4_sb = cpool.tile([1, DIM], F32)
            nc.scalar.dma_start(bo4_sb, bo4[:, :])
            bmask_sb = cpool.tile([HID, HID], F32)
            nc.scalar.dma_start(bmask_sb, bmask[:, :])
            ones1_sb = cpool.tile([1, HID], F32)
            nc.gpsimd.memset(ones1_sb, 1.0)

            for i in range(n_tiles + 1):
                issue_loads(i + 2)

                if i < n_tiles:
                    ct = cts.pop(i)
                    # kv projection: n on partitions, 2 chunk-groups of 2
                    for g in range(chunks // 2):
                        kvt_ps = ps_kv.tile([128, 2, 2 * HID], F32, tag="kvt")
                        for j2 in range(2):
                            j = g * 2 + j2
                            for cc in range(4):
                                nc.tensor.matmul(
                                    kvt_ps[:, j2, :],
                                    ct[:, cc, j * 128 : (j + 1) * 128],
                                    wkvt_sb[:, cc, :],
                                    start=(cc == 0),
                                    stop=(cc == 3),
                                )
                        kt = kvpool.tile([128, 2, HID], BF16, tag="kt")
                        nc.scalar.activation(kt, kvt_ps[:, :, 0:HID], EXP)
                        vto = kvpool.tile([128, 2, HID + 4], BF16, tag="vto")
                        nc.gpsimd.memset(vto[:, :, HID : HID + 1], 1.0)
                        nc.vector.tensor_copy(
                            vto[:, :, 0:HID], kvt_ps[:, :, HID : 2 * HID]
                        )
                        for j2 in range(2):
                            ci = i * chunks + g * 2 + j2
                            nc.tensor.matmul(
                                cz_ps,
                                kt[:, j2, :],
                                vto[:, j2, 0 : HID + 1],
                                start=(ci == 0),
                                stop=(ci == total_chunks - 1),
                            )

                    # q projection + exp for tile i
                    xt = xts.pop(i)
                    q_ps = ps_q.tile([128, TILE_N], F32, tag="q")
                    for cc in range(2):
                        nc.tensor.matmul(
                            q_ps,
                            wqt_sb[:, cc, :],
                            xt[:, cc, :],
                            start=(cc == 0),
                            stop=(cc == 1),
                        )
                    nc.scalar.activation(eq_all[:, i, :], q_ps, EXP)

                if 1 <= i <= n_tiles:
                    # per-head sums + 1/S for tile i-1
                    t = i - 1
                    s_ps = ps_s.tile([HEADS, TILE_N], F32, tag="s")
                    nc.tensor.matmul(
                        s_ps, ind4_sb, eq_all[:, t, :], start=True, stop=True
                    )
                    rsf = rspool.tile([HEADS, TILE_N], F32, tag="rsf")
                    nc.vector.reciprocal_approx_fast(rsf, s_ps)
                    nc.gpsimd.tensor_copy(rs_all[:, t, :], rsf)

                if i >= 2 and (i - 2) % 2 == 0:
                    # normalize EVEN tile i-2 here; odd tiles in phase 2
                    t = i - 2
                    rb_ps = ps_rb1.tile([128, TILE_N], F32, tag="rb1")
                    nc.tensor.matmul(
                        rb_ps, ind128_sb, rs_all[:, t, :], start=True, stop=True
                    )
                    nc.vector.tensor_mul(
                        qn_even[:, t // 2, :], eq_all[:, t, :], rb_ps
                    )

            # ------- Weff fold: WeffT = (C.bmask/Z)^T @ WoT + ones.bo/4 ----
            rz = kvpool.tile([128, 1], F32, tag="rz")
            nc.vector.reciprocal(rz, cz_ps[:, HID : HID + 1])
            cm0 = kvpool.tile([128, HID], F32, tag="cm0")
            nc.vector.tensor_scalar_mul(cm0, cz_ps[:, 0:HID], rz)
            cmask = kvpool.tile([128, HID], F32, tag="cmask")
            nc.vector.tensor_mul(cmask, cm0, bmask_sb)
            cmaskT = kvpool.tile([128, HID], F32, tag="cmaskT")
            # block-diagonal 32x32 in-place transposes == full C^T here
            nc.vector.transpose(cmaskT, cmask)

            weff_ps = ps_cz.tile([128, DIM], F32, tag="weff")
            nc.tensor.matmul(weff_ps, ones1_sb, bo4_sb, start=True, stop=False)
            nc.tensor.matmul(weff_ps, cmaskT, wot_sb, start=False, stop=True)
            nc.vector.tensor_copy(weffT_sb, weff_ps)

        # ------- Phase 2: bcast 1/S, qn = eq*rb, y = WeffT.T @ qn -----------
        with (
            tc.tile_pool(name="ytp", bufs=6) as ytpool,
            tc.tile_pool(name="qnp", bufs=4) as qnpool,
            tc.tile_pool(name="ps_rb", bufs=2, space="PSUM") as ps_rb,
            tc.tile_pool(name="ps_y", bufs=3, space="PSUM") as ps_y,
        ):
            qns = {}

            def bcast(t):
                # broadcast 1/S for ODD tile t (evens were done in phase 1)
                if t >= n_tiles:
                    return None
                rb_ps = ps_rb.tile([128, TILE_N], F32, tag="rb")
                nc.tensor.matmul(
                    rb_ps, ind128_sb, rs_all[:, t, :], start=True, stop=True
                )
                return rb_ps

            def qn_mul(t, rb_ps):
                if rb_ps is None:
                    return
                qn = qnpool.tile([128, TILE_N], BF16, tag="qn")
                nc.vector.tensor_mul(qn, eq_all[:, t, :], rb_ps)
                qns[t] = qn

            rb1 = bcast(1)
            qn_mul(1, rb1)
            for t in range(n_tiles):
                tsl = slice(t * TILE_N, (t + 1) * TILE_N)
                # PE: prefetch the next odd tile's broadcast ahead of y(t)
                rb_next = bcast(t + 2) if t % 2 == 1 else None
                y_ps = ps_y.tile([128, 2, TILE_N], F32, tag="y")
                qn = qns.pop(t) if t % 2 == 1 else qn_even[:, t // 2, :]
                for oc in range(2):
                    nc.tensor.matmul(
                        y_ps[:, oc, :],
                        weffT_sb[:, oc * 128 : (oc + 1) * 128],
                        qn,
                        start=True,
                        stop=True,
                    )
                yt = ytpool.tile([128, 2, TILE_N], BF16, tag="yt")
                nc.scalar.activation(yt[:, 0, :], y_ps[:, 0, :], COPY)
                nc.vector.tensor_copy(yt[:, 1, :], y_ps[:, 1, :])
                if rb_next is not None:
                    qn_mul(t + 2, rb_next)
                nc.sync.dma_start(yr[:, :, tsl], yt)

    nc.compile()
    return nc


_GRAPH_CACHE: dict = {}


def _prep_inputs(x, context, Wq, Wkv, Wo, bo):
    bf16 = ml_dtypes.bfloat16
    x = np.asarray(x, dtype=np.float32)
    context = np.asarray(context, dtype=np.float32)
    Wq = np.asarray(Wq, dtype=np.float32)
    Wkv = np.asarray(Wkv, dtype=np.float32)
    Wo = np.asarray(Wo, dtype=np.float32)
    bo = np.asarray(bo, dtype=np.float32)

    wqt = np.ascontiguousarray(Wq.T).astype(bf16)              # (256, 128)
    wkvt = np.ascontiguousarray(Wkv.T).astype(bf16)            # (512, 256)
    # SCALE folded into Wo; kept f32 for the one-shot Weff matmul
    wot = np.ascontiguousarray((Wo * SCALE).T).astype(np.float32)  # (128, 256)
    # rows of qn sum to exactly 4 -> rank-1 bias fold
    bo4 = np.ascontiguousarray((bo / 4.0)[None, :]).astype(np.float32)  # (1, 256)

    ind4 = np.zeros((HID, HEADS), dtype=bf16)
    ind4[np.arange(HID), np.arange(HID) // DIM_HEAD] = 1
    ind128 = np.ascontiguousarray(ind4.T)
    bmask = (
        (np.arange(HID)[:, None] // DIM_HEAD) == (np.arange(HID)[None, :] // DIM_HEAD)
    ).astype(np.float32)

    in_maps = []
    for b in range(B):
        in_maps.append(
            {
                "ctxt": np.ascontiguousarray(context[b].T).astype(bf16),
                "xs": x[b].astype(bf16),
                "wqt": wqt,
                "wkvt": wkvt,
                "wot": wot,
                "bo4": bo4,
                "ind4": ind4,
                "ind128": ind128,
                "bmask": bmask,
            }
        )
    return in_maps


def run(inputs: dict, trace: bool = False):
    if "nc" not in _GRAPH_CACHE:
        _GRAPH_CACHE["nc"] = build_graph()
    nc = _GRAPH_CACHE["nc"]
    in_maps = _prep_inputs(**inputs)
    res = run_bass_kernel_spmd(nc, in_maps, core_ids=list(range(B)), trace=trace)
    out = np.stack(
        [np.asarray(res.results[b]["y"], dtype=np.float32) for b in range(B)]
    )
    return out, res


def kernel(**inputs) -> np.ndarray:
    out, _ = run(inputs, trace=False)
    return out
